# revision 25
# baseline (speedup 1.0000x reference)
"""Trainium2 Bass kernel for an AttentionBlock (InstanceNorm + single-head
spatial self-attention + projection + residual).

Full-input contract: kernel(**inputs) takes the complete tensors and returns
the complete output. Internally shards across 8 NeuronCores: data-parallel
over batch (B=4 -> 4 pairs of cores), sequence-parallel over the N=4096 query
positions within each sample (2 cores per sample, 2048 queries each).

All 8 cores run the *same* program; the query-half assignment is done by
rotating the spatial columns of x host-side (attention and instance-norm
statistics are invariant under column permutation).

v4 design (v3 + engine rebalance):
  - pass-1 ([q,k] layout): f16 matmul pairs per (q-tile, 2 k-chunks); DVE
    tensor_tensor_reduce fuses the elementwise max of the two PSUM tiles
    with the row-max accumulation (half the DVE passes of per-tile reduce).
  - pass-2 ([k,q] layout): K=65 matmul (64 channels + bias row carrying
    -rowmax) produces shifted scores in PSUM; ScalarE exp writes f16
    attention blocks straight to SBUF in the layout attn@v consumes.
  - attn@v: v is augmented with a ones column so softmax denominators fall
    out of the same accumulation.
  - epilogue: one K=65 matmul with wo1=[[WoT,0],[0,1...1]] yields both the
    output projection (rows 0:64) and the denominator broadcast to rows
    64:128; reciprocal+normalize then run at full DVE width.
  - copies ride GpSimd (Pool) where possible to keep ScalarE free for exp.
  - no PE warm-up/dummy matmuls: the HAM power throttle (k=4/8 duty) is
    triggered by sustained PE activity, so wasted matmuls cost twice.
"""

import os
import sys
import numpy as np
from contextlib import ExitStack

for _p in ("/opt/trn_rl_repo", "/root/.axon_site/_ro/trn_rl_repo"):
    if os.path.isdir(_p) and _p not in sys.path:
        sys.path.append(_p)

from concourse import bass, bacc, tile, mybir, masks  # noqa: E402
from concourse.bass_utils import run_bass_kernel_spmd  # noqa: E402

F32 = mybir.dt.float32
F16 = mybir.dt.float16
BF16 = mybir.dt.bfloat16
F8 = mybir.dt.float8e4

B, C, H, W = 4, 64, 64, 64
N = H * W            # 4096 spatial positions (attention length)
HALF = N // 2        # queries per core
KT = 128             # pass-2 k-tile (partition dim of transposed scores)
NKT = N // KT        # 32 k-tiles
NPR = NKT // 2       # 16 k-tile pairs
QC = 512             # q-chunk (PSUM bank free dim)
NQC = HALF // QC     # 4 q-chunks per core
QT = 128             # pass-1 q-tile
KC = 512             # pass-1 k-chunk
NKC = N // KC        # 8
EPS = 1e-5
NCORES = 8
NEG_INF = -3.0e38
USE_FP8_P1 = os.environ.get("USE_FP8_P1", "1") == "1"
USE_RECIP_APPROX = os.environ.get("USE_RECIP_APPROX", "0") == "1"


def build_nc():
    nc = bacc.Bacc("TRN2", target_bir_lowering=False, debug=False)

    x_d = nc.dram_tensor("x", [C, N], F32, kind="ExternalInput")
    wq_d = nc.dram_tensor("wq1", [C + 1, 2, C], F16, kind="ExternalInput")
    wk_d = nc.dram_tensor("wk1", [C + 1, 2, C], F16, kind="ExternalInput")
    wv_d = nc.dram_tensor("wv1", [C + 1, 2, C], F16, kind="ExternalInput")
    wo_d = nc.dram_tensor("wo1", [C + 1, KT], BF16, kind="ExternalInput")
    bo_d = nc.dram_tensor("bo", [C, 1], F32, kind="ExternalInput")
    out_d = nc.dram_tensor("out", [C, HALF], F32, kind="ExternalOutput")

    with tile.TileContext(nc) as tc:
        _body(tc, x_d, wq_d, wk_d, wv_d, wo_d, bo_d, out_d)
    nc.compile()
    return nc


def _body(tc, x_d, wq_d, wk_d, wv_d, wo_d, bo_d, out_d):
    nc = tc.nc
    with ExitStack() as ctx:
        persist = ctx.enter_context(tc.tile_pool(name="persist", bufs=1))
        small = ctx.enter_context(tc.tile_pool(name="small", bufs=4))
        apool = ctx.enter_context(tc.tile_pool(name="apool", bufs=6))
        fpool = ctx.enter_context(tc.tile_pool(name="fpool", bufs=2))
        # PSUM budget (8 banks): p1p 2 + scp 4 + avp 2
        p1p = ctx.enter_context(tc.tile_pool(name="p1p", bufs=2, space="PSUM"))
        scp = ctx.enter_context(tc.tile_pool(name="scp", bufs=2, space="PSUM"))
        avp = ctx.enter_context(tc.tile_pool(name="avp", bufs=2, space="PSUM"))

        # ---- inputs ----
        x_sb = persist.tile([C, N], F32)
        wq_sb = persist.tile([C + 1, 2, C], F16)
        nc.sync.dma_start(out=wq_sb, in_=wq_d.ap())
        wk_sb = persist.tile([C + 1, 2, C], F16)
        nc.scalar.dma_start(out=wk_sb, in_=wk_d.ap())
        wv_sb = persist.tile([C + 1, 2, C], F16)
        nc.sync.dma_start(out=wv_sb, in_=wv_d.ap())
        wo_sb = persist.tile([C + 1, KT], BF16)
        nc.scalar.dma_start(out=wo_sb, in_=wo_d.ap())
        bo_sb = persist.tile([C, 1], F32)
        nc.sync.dma_start(out=bo_sb, in_=bo_d.ap())
        eps_t = persist.tile([C, 1], F32)
        nc.vector.memset(eps_t, EPS)
        ident = persist.tile([QT, QT], F16)
        masks.make_identity(nc, ident)

        # x load split across queues; bn_stats per chunk as it arrives
        stats = persist.tile([C, NKC, nc.vector.BN_STATS_DIM], F32)
        for i in range(NKC):
            sl = slice(i * KC, (i + 1) * KC)
            eng = nc.sync if i % 2 == 0 else nc.scalar
            eng.dma_start(out=x_sb[:, sl], in_=x_d.ap()[:, sl])
            nc.vector.bn_stats(out=stats[:, i, :], in_=x_sb[:, sl])
        mv = persist.tile([C, nc.vector.BN_AGGR_DIM], F32)
        nc.vector.bn_aggr(out=mv, in_=stats)
        stdv = persist.tile([C, 1], F32)
        nc.scalar.activation(out=stdv, in_=mv[:, 1:2],
                             func=mybir.ActivationFunctionType.Sqrt,
                             bias=eps_t, scale=1.0)
        rstd = persist.tile([C, 1], F32)
        nc.vector.reciprocal(out=rstd, in_=stdv)
        nmr = persist.tile([C, 1], F32)
        nc.vector.tensor_mul(nmr, mv[:, 0:1], rstd)
        nc.vector.tensor_scalar_mul(nmr, nmr, -1.0)

        # xn (f32, residual + lo-part), f16 hi/lo with bias-row for QKV.
        # Per 512-chunk: xn on DVE, xnh on ScalarE, xnl on GpSimd, then the
        # k-projection for that chunk immediately (keeps the PE fed early).
        xn = persist.tile([C, N], F32)
        xnh = persist.tile([C + 1, N], F16)
        xnl = persist.tile([C + 1, N], F16)
        nc.gpsimd.memset(xnh[C:C + 1, :], 1.0)
        nc.gpsimd.memset(xnl[C:C + 1, :], 0.0)
        kst = persist.tile([C + 1, N], F16)      # rows 0:64 k, row 64 ones
        qrhs = persist.tile([C + 1, HALF], F16)  # rows 0:64 q*sqrt(C), row 64 -max
        nc.gpsimd.memset(kst[C:C + 1, :], 1.0)
        vst = persist.tile([KT, NKT, 66], BF16)  # [kpos, ktile, 64 v + ones + pad]
        nc.gpsimd.memset(vst[:, :, 64:65], 1.0)
        nc.gpsimd.memset(vst[:, :, 65:66], 0.0)
        # fp8 copies of k and q*sqrt(C), channel-split [32, 2, *] for the
        # DoubleRow pass-1 matmuls (2 contraction sub-tiles per pass).
        k8 = persist.tile([C // 2, 2, N], F8)
        q8 = persist.tile([C // 2, 2, HALF], F8)

        for i in range(NKC):
            sl = slice(i * KC, (i + 1) * KC)
            nc.vector.tensor_scalar(out=xn[:, sl], in0=x_sb[:, sl],
                                    scalar1=rstd, scalar2=nmr,
                                    op0=mybir.AluOpType.mult,
                                    op1=mybir.AluOpType.add)
            nc.scalar.activation(out=xnh[0:C, sl], in_=x_sb[:, sl],
                                 func=mybir.ActivationFunctionType.Identity,
                                 bias=nmr, scale=rstd)
            nc.gpsimd.tensor_sub(xnl[0:C, sl], xn[:, sl], xnh[0:C, sl])
            kp = scp.tile([KT, 2, KC], F32, tag="sc", name=f"kp{i}")
            nc.tensor.matmul(kp[0:C, 0, :], lhsT=wk_sb[:, 0, :], rhs=xnh[:, sl],
                             start=True, stop=False, skip_group_check=True)
            nc.tensor.matmul(kp[0:C, 0, :], lhsT=wk_sb[:, 0, :], rhs=xnl[:, sl],
                             start=False, stop=True, skip_group_check=True)
            if i % 2 == 0:
                nc.vector.tensor_copy(kst[0:C, sl], kp[0:C, 0, :])
            else:
                nc.scalar.copy(kst[0:C, sl], kp[0:C, 0, :])
            # fp8 casts: sub-tile 0 partition-aligned (GpSimd), sub-tile 1
            # crosses partition base 32->0 (ScalarE handles offset bases).
            nc.gpsimd.tensor_copy(k8[:, 0, sl], kst[0:C // 2, sl])
            nc.scalar.copy(k8[:, 1, sl], kst[C // 2:C, sl])
        # residual + output bias for our query half
        xnb = persist.tile([C, HALF], F32)
        nc.vector.tensor_scalar_add(xnb, xn[:, 0:HALF], bo_sb)

        for i in range(NQC):
            sl = slice(i * QC, (i + 1) * QC)
            qp = scp.tile([KT, 2, KC], F32, tag="sc", name=f"qp{i}")
            nc.tensor.matmul(qp[0:C, 0, :], lhsT=wq_sb[:, 0, :], rhs=xnh[:, sl],
                             start=True, stop=False, skip_group_check=True)
            nc.tensor.matmul(qp[0:C, 0, :], lhsT=wq_sb[:, 0, :], rhs=xnl[:, sl],
                             start=False, stop=True, skip_group_check=True)
            if i % 2 == 0:
                nc.vector.tensor_copy(qrhs[0:C, sl], qp[0:C, 0, :])
            else:
                nc.scalar.copy(qrhs[0:C, sl], qp[0:C, 0, :])
            nc.gpsimd.tensor_copy(q8[:, 0, sl], qrhs[0:C // 2, sl])
            nc.scalar.copy(q8[:, 1, sl], qrhs[C // 2:C, sl])

        # ---- incremental pass-1 (row max of chunk c1's q-tiles) ----
        # One k-chunk matmul + DVE row-max per step; every 8th step finalizes
        # a q-tile's -max into qrhs row 64 via a PE transpose. (DVE can read
        # only ONE PSUM operand per instruction, so per-chunk tensor_reduce
        # it is.)
        p1_state = {}

        def pass1_step(c1):
            st = p1_state.setdefault(c1, {"step": 0})
            step = st["step"]
            if step >= 4 * NKC:
                return
            st["step"] = step + 1
            t4, ci = divmod(step, NKC)
            t = c1 * 4 + t4
            tq = slice(t * QT, (t + 1) * QT)
            if ci == 0:
                st["cm"] = small.tile([QT, NKC], F32, tag="cm",
                                      name=f"cm{t}")
            cm = st["cm"]
            cs = slice(ci * KC, (ci + 1) * KC)
            p1 = p1p.tile([QT, KC], F32, tag="p1", name=f"p1_{t}_{ci}")
            if USE_FP8_P1:
                nc.tensor.matmul(p1, lhsT=q8[:, :, tq], rhs=k8[:, :, cs],
                                 perf_mode=mybir.MatmulPerfMode.DoubleRow,
                                 start=True, stop=True, skip_group_check=True)
            else:
                nc.tensor.matmul(p1, lhsT=qrhs[0:C, tq], rhs=kst[0:C, cs],
                                 start=True, stop=True, skip_group_check=True)
            nc.vector.tensor_reduce(cm[:, ci:ci + 1], p1,
                                    axis=mybir.AxisListType.X,
                                    op=mybir.AluOpType.max)
            if ci == NKC - 1:
                nmT = small.tile([QT, 1], F16, tag="nmT", name=f"nmT{t}")
                nc.vector.tensor_reduce(nmT, cm,
                                        axis=mybir.AxisListType.X,
                                        op=mybir.AluOpType.max, negate=True)
                tr = p1p.tile([1, QT], F16, tag="p1", name=f"tr{t}")
                nc.tensor.transpose(tr, nmT, ident)
                nc.scalar.copy(qrhs[C:C + 1, tq], tr[0:1, :])

        # v projection interleaved with chunk-0 pass-1
        for j in range(NKT):
            js = slice(j * KT, (j + 1) * KT)
            vp = p1p.tile([KT, C], F32, tag="p1", name=f"vp{j}")
            nc.tensor.matmul(vp, lhsT=xnh[:, js], rhs=wv_sb[:, 0, :],
                             start=True, stop=False, skip_group_check=True)
            nc.tensor.matmul(vp, lhsT=xnl[:, js], rhs=wv_sb[:, 0, :],
                             start=False, stop=True, skip_group_check=True)
            if j % 2 == 0:
                nc.scalar.copy(vst[:, j, 0:C], vp)
            else:
                nc.vector.tensor_copy(vst[:, j, 0:C], vp)
            pass1_step(0)

        # ---- main loop over q-chunks ----
        # attn@v matmuls trail the score/exp conveyor by AV_LAG pairs so they
        # never head-of-line block the in-order PE queue on a fresh exp.
        ao_aug = persist.tile([C + 1, HALF], BF16)  # rows 0:64 attn@v, 64 denom
        AV_LAG = 3
        av_fifo = []

        def emit_av(c, p, otp, ab):
            for h in range(2):
                j = 2 * p + h
                nc.tensor.matmul(otp, lhsT=vst[:, j, :], rhs=ab[:, h, :],
                                 start=(j == 0), stop=(j == NKT - 1),
                                 skip_group_check=True)

        def emit_epilogue(c, otp):
            qs = slice(c * QC, (c + 1) * QC)
            # single copy moves attn@v rows AND the denominator row; DVE
            # keeps it off the exp-laden ScalarE queue so the fx matmul
            # doesn't head-of-line block the PE behind pending exps.
            # (GPSIMD cannot read PSUM on TRN2.)
            nc.vector.tensor_copy(ao_aug[:, qs], otp[0:C + 1, :])
            # fx rows 0:64 = Wo @ attn@v ; rows 64:128 = denominator bcast
            fx = p1p.tile([KT, QC], F32, tag="p1", name=f"fx{c}")
            nc.tensor.matmul(fx, lhsT=wo_sb, rhs=ao_aug[:, qs],
                             start=True, stop=True, skip_group_check=True)
            ibs = fpool.tile([C, QC], F32, tag="ibs", name=f"ibs{c}")
            if USE_RECIP_APPROX:
                nc.vector.reciprocal_approx_fast(out=ibs, in_=fx[C:2 * C, :])
            else:
                nc.vector.reciprocal(out=ibs, in_=fx[C:2 * C, :])
            fin = fpool.tile([C, QC], F32, tag="fin", name=f"fin{c}")
            nc.vector.tensor_mul(fin, fx[0:C, :], ibs)
            nc.vector.tensor_add(fin, fin, xnb[:, qs])
            eng = nc.sync if c % 2 == 0 else nc.scalar
            eng.dma_start(out=out_d.ap()[:, qs], in_=fin)

        def pop_av():
            c0, p0, otp0, ab0 = av_fifo.pop(0)
            emit_av(c0, p0, otp0, ab0)
            if p0 == NPR - 1:
                emit_epilogue(c0, otp0)

        for c in range(NQC):
            qs = slice(c * QC, (c + 1) * QC)
            otp = avp.tile([66, QC], F32, tag="av", name=f"otp{c}")
            for p in range(NPR):
                sc = scp.tile([KT, 2, QC], F32, tag="sc", name=f"sc{c}_{p}")
                for h in range(2):
                    js = slice((2 * p + h) * KT, (2 * p + h + 1) * KT)
                    nc.tensor.matmul(sc[:, h, :], lhsT=kst[:, js],
                                     rhs=qrhs[:, qs],
                                     start=True, stop=True,
                                     skip_group_check=True)
                ab = apool.tile([KT, 2, QC], BF16, tag="ab", name=f"ab{c}_{p}")
                nc.scalar.activation(out=ab.rearrange("p a b -> p (a b)"),
                                     in_=sc.rearrange("p a b -> p (a b)"),
                                     func=mybir.ActivationFunctionType.Exp,
                                     bias=0.0, scale=1.0)
                av_fifo.append((c, p, otp, ab))
                if len(av_fifo) > AV_LAG:
                    pop_av()
                if c + 1 < NQC:
                    # 3 steps/pair drains the 32 steps by pair 11 so the
                    # next chunk's -max bias lands well before the boundary.
                    pass1_step(c + 1)
                    pass1_step(c + 1)
                    pass1_step(c + 1)
        while av_fifo:
            pop_av()


def prep_inputs(x, w_qkv, b_qkv, w_out, b_out):
    """Host-side slicing/packing into per-core input maps."""
    x = np.asarray(x, dtype=np.float32).reshape(B, C, N)
    w_qkv = np.asarray(w_qkv, dtype=np.float32)
    b_qkv = np.asarray(b_qkv, dtype=np.float32)
    w_out = np.asarray(w_out, dtype=np.float32)
    b_out = np.asarray(b_out, dtype=np.float32)

    s = float(C) ** 0.5  # reference multiplies scores by sqrt(C)
    wq1 = np.concatenate([s * w_qkv[0:C].T, s * b_qkv[None, 0:C]], axis=0)
    wk1 = np.concatenate([w_qkv[C:2 * C].T, b_qkv[None, C:2 * C]], axis=0)
    wv1 = np.concatenate([w_qkv[2 * C:3 * C].T, b_qkv[None, 2 * C:3 * C]], axis=0)

    def hilo16(w):  # [65, 64] -> [65, 2, 64] f16 (hi, lo), hi+lo ~== w
        hi = w.astype(np.float16)
        lo = (w - hi.astype(np.float32)).astype(np.float16)
        return np.ascontiguousarray(np.stack([hi, lo], axis=1))

    wq1 = hilo16(np.ascontiguousarray(wq1))
    wk1 = hilo16(np.ascontiguousarray(wk1))
    wv1 = hilo16(np.ascontiguousarray(wv1))
    # wo1: [65, 128]; rows 0:64 cols 0:64 = WoT; row 64 cols 64:128 = 1
    # so one K=65 matmul gives [Wo@ao ; denom broadcast] stacked.
    import ml_dtypes
    wo1 = np.zeros((C + 1, KT), dtype=np.float32)
    wo1[0:C, 0:C] = w_out.T
    wo1[C, C:KT] = 1.0
    wo1 = np.ascontiguousarray(wo1).astype(ml_dtypes.bfloat16)
    bo = np.ascontiguousarray(b_out[:, None])

    in_maps = []
    for j in range(NCORES):
        b, h = divmod(j, 2)
        xs = x[b]
        if h == 1:
            xs = np.concatenate([xs[:, HALF:], xs[:, :HALF]], axis=1)
        in_maps.append({
            "x": np.ascontiguousarray(xs),
            "wq1": wq1,
            "wk1": wk1,
            "wv1": wv1,
            "wo1": wo1,
            "bo": bo,
        })
    return in_maps


def gather_output(results):
    out = np.empty((B, C, N), dtype=np.float32)
    for j in range(NCORES):
        b, h = divmod(j, 2)
        out[b][:, h * HALF:(h + 1) * HALF] = results[j]["out"]
    return out.reshape(B, C, H, W)


_NC_CACHE = {}


def get_nc():
    key = "v5"
    if key not in _NC_CACHE:
        _NC_CACHE[key] = build_nc()
    return _NC_CACHE[key]


def kernel(x, w_qkv, b_qkv, w_out, b_out):
    nc = get_nc()
    in_maps = prep_inputs(x, w_qkv, b_qkv, w_out, b_out)
    res = run_bass_kernel_spmd(nc, in_maps, list(range(NCORES)))
    return gather_output(res.results)


# revision 29
# speedup vs baseline: 1.0078x; 1.0078x over previous
"""Trainium2 Bass kernel for an AttentionBlock (InstanceNorm + single-head
spatial self-attention + projection + residual).

Full-input contract: kernel(**inputs) takes the complete tensors and returns
the complete output. Internally shards across 8 NeuronCores: data-parallel
over batch (B=4 -> 4 pairs of cores), sequence-parallel over the N=4096 query
positions within each sample (2 cores per sample, 2048 queries each).

All 8 cores run the *same* program; the query-half assignment is done by
rotating the spatial columns of x host-side (attention and instance-norm
statistics are invariant under column permutation).

v4 design (v3 + engine rebalance):
  - pass-1 ([q,k] layout): f16 matmul pairs per (q-tile, 2 k-chunks); DVE
    tensor_tensor_reduce fuses the elementwise max of the two PSUM tiles
    with the row-max accumulation (half the DVE passes of per-tile reduce).
  - pass-2 ([k,q] layout): K=65 matmul (64 channels + bias row carrying
    -rowmax) produces shifted scores in PSUM; ScalarE exp writes f16
    attention blocks straight to SBUF in the layout attn@v consumes.
  - attn@v: v is augmented with a ones column so softmax denominators fall
    out of the same accumulation.
  - epilogue: one K=65 matmul with wo1=[[WoT,0],[0,1...1]] yields both the
    output projection (rows 0:64) and the denominator broadcast to rows
    64:128; reciprocal+normalize then run at full DVE width.
  - copies ride GpSimd (Pool) where possible to keep ScalarE free for exp.
  - no PE warm-up/dummy matmuls: the HAM power throttle (k=4/8 duty) is
    triggered by sustained PE activity, so wasted matmuls cost twice.
"""

import os
import sys
import numpy as np
from contextlib import ExitStack

for _p in ("/opt/trn_rl_repo", "/root/.axon_site/_ro/trn_rl_repo"):
    if os.path.isdir(_p) and _p not in sys.path:
        sys.path.append(_p)

from concourse import bass, bacc, tile, mybir, masks  # noqa: E402
from concourse.bass_utils import run_bass_kernel_spmd  # noqa: E402

F32 = mybir.dt.float32
F16 = mybir.dt.float16
BF16 = mybir.dt.bfloat16
F8 = mybir.dt.float8e4

B, C, H, W = 4, 64, 64, 64
N = H * W            # 4096 spatial positions (attention length)
HALF = N // 2        # queries per core
KT = 128             # pass-2 k-tile (partition dim of transposed scores)
NKT = N // KT        # 32 k-tiles
NPR = NKT // 2       # 16 k-tile pairs
QC = 512             # q-chunk (PSUM bank free dim)
NQC = HALF // QC     # 4 q-chunks per core
QT = 128             # pass-1 q-tile
KC = 512             # pass-1 k-chunk
NKC = N // KC        # 8
EPS = 1e-5
NCORES = 8
NEG_INF = -3.0e38
USE_FP8_P1 = os.environ.get("USE_FP8_P1", "1") == "1"
USE_RECIP_APPROX = os.environ.get("USE_RECIP_APPROX", "0") == "1"


def build_nc():
    nc = bacc.Bacc("TRN2", target_bir_lowering=False, debug=False)

    x_d = nc.dram_tensor("x", [C, N], F32, kind="ExternalInput")
    wq_d = nc.dram_tensor("wq1", [C + 1, 2, C], F16, kind="ExternalInput")
    wk_d = nc.dram_tensor("wk1", [C + 1, 2, C], F16, kind="ExternalInput")
    wv_d = nc.dram_tensor("wv1", [C + 1, 2, C], F16, kind="ExternalInput")
    wo_d = nc.dram_tensor("wo1", [C + 1, KT], BF16, kind="ExternalInput")
    bo_d = nc.dram_tensor("bo", [C, 1], F32, kind="ExternalInput")
    out_d = nc.dram_tensor("out", [C, HALF], F32, kind="ExternalOutput")

    with tile.TileContext(nc) as tc:
        _body(tc, x_d, wq_d, wk_d, wv_d, wo_d, bo_d, out_d)
    nc.compile()
    return nc


def _body(tc, x_d, wq_d, wk_d, wv_d, wo_d, bo_d, out_d):
    nc = tc.nc
    with ExitStack() as ctx:
        persist = ctx.enter_context(tc.tile_pool(name="persist", bufs=1))
        small = ctx.enter_context(tc.tile_pool(name="small", bufs=4))
        apool = ctx.enter_context(tc.tile_pool(name="apool", bufs=6))
        fpool = ctx.enter_context(tc.tile_pool(name="fpool", bufs=2))
        # PSUM budget (8 banks): p1p 2 + scp 4 + avp 2
        p1p = ctx.enter_context(tc.tile_pool(name="p1p", bufs=2, space="PSUM"))
        scp = ctx.enter_context(tc.tile_pool(name="scp", bufs=2, space="PSUM"))
        avp = ctx.enter_context(tc.tile_pool(name="avp", bufs=2, space="PSUM"))

        # ---- inputs ----
        x_sb = persist.tile([C, N], F32)
        wq_sb = persist.tile([C + 1, 2, C], F16)
        nc.sync.dma_start(out=wq_sb, in_=wq_d.ap())
        wk_sb = persist.tile([C + 1, 2, C], F16)
        nc.scalar.dma_start(out=wk_sb, in_=wk_d.ap())
        wv_sb = persist.tile([C + 1, 2, C], F16)
        nc.sync.dma_start(out=wv_sb, in_=wv_d.ap())
        wo_sb = persist.tile([C + 1, KT], BF16)
        nc.scalar.dma_start(out=wo_sb, in_=wo_d.ap())
        bo_sb = persist.tile([C, 1], F32)
        nc.sync.dma_start(out=bo_sb, in_=bo_d.ap())
        eps_t = persist.tile([C, 1], F32)
        nc.vector.memset(eps_t, EPS)
        ident = persist.tile([QT, QT], F16)
        masks.make_identity(nc, ident)
        dmy = persist.tile([QT, KC], F16)
        nc.gpsimd.memset(dmy, 0.25)

        def warm_pe(tag, n):
            """Back-to-back dummy matmuls (no readers -> no stalls) that keep
            the PE queue dense through a full HAM window so the clock gate
            opens to 8/8 (2.4 GHz)."""
            for w in range(n):
                wp = p1p.tile([QT, KC], F32, tag="p1", name=f"wp{tag}_{w}")
                nc.tensor.matmul(wp, lhsT=ident, rhs=dmy,
                                 start=True, stop=True, skip_group_check=True)

        # x load split across queues; bn_stats per chunk as it arrives
        stats = persist.tile([C, NKC, nc.vector.BN_STATS_DIM], F32)
        warm_pe("h", 12)
        for i in range(NKC):
            sl = slice(i * KC, (i + 1) * KC)
            eng = nc.sync if i % 2 == 0 else nc.scalar
            eng.dma_start(out=x_sb[:, sl], in_=x_d.ap()[:, sl])
            nc.vector.bn_stats(out=stats[:, i, :], in_=x_sb[:, sl])
        mv = persist.tile([C, nc.vector.BN_AGGR_DIM], F32)
        nc.vector.bn_aggr(out=mv, in_=stats)
        stdv = persist.tile([C, 1], F32)
        nc.scalar.activation(out=stdv, in_=mv[:, 1:2],
                             func=mybir.ActivationFunctionType.Sqrt,
                             bias=eps_t, scale=1.0)
        rstd = persist.tile([C, 1], F32)
        nc.vector.reciprocal(out=rstd, in_=stdv)
        nmr = persist.tile([C, 1], F32)
        nc.vector.tensor_mul(nmr, mv[:, 0:1], rstd)
        nc.vector.tensor_scalar_mul(nmr, nmr, -1.0)

        # xn (f32, residual + lo-part), f16 hi/lo with bias-row for QKV.
        # Per 512-chunk: xn on DVE, xnh on ScalarE, xnl on GpSimd, then the
        # k-projection for that chunk immediately (keeps the PE fed early).
        xn = persist.tile([C, N], F32)
        xnh = persist.tile([C + 1, N], F16)
        xnl = persist.tile([C + 1, N], F16)
        nc.gpsimd.memset(xnh[C:C + 1, :], 1.0)
        nc.gpsimd.memset(xnl[C:C + 1, :], 0.0)
        kst = persist.tile([C + 1, N], F16)      # rows 0:64 k, row 64 ones
        qrhs = persist.tile([C + 1, HALF], F16)  # rows 0:64 q*sqrt(C), row 64 -max
        nc.gpsimd.memset(kst[C:C + 1, :], 1.0)
        vst = persist.tile([KT, NKT, 66], BF16)  # [kpos, ktile, 64 v + ones + pad]
        nc.gpsimd.memset(vst[:, :, 64:65], 1.0)
        nc.gpsimd.memset(vst[:, :, 65:66], 0.0)
        # fp8 copies of k and q*sqrt(C), channel-split [32, 2, *] for the
        # DoubleRow pass-1 matmuls (2 contraction sub-tiles per pass).
        k8 = persist.tile([C // 2, 2, N], F8)
        q8 = persist.tile([C // 2, 2, HALF], F8)

        for i in range(NKC):
            sl = slice(i * KC, (i + 1) * KC)
            nc.vector.tensor_scalar(out=xn[:, sl], in0=x_sb[:, sl],
                                    scalar1=rstd, scalar2=nmr,
                                    op0=mybir.AluOpType.mult,
                                    op1=mybir.AluOpType.add)
            nc.scalar.activation(out=xnh[0:C, sl], in_=x_sb[:, sl],
                                 func=mybir.ActivationFunctionType.Identity,
                                 bias=nmr, scale=rstd)
            nc.gpsimd.tensor_sub(xnl[0:C, sl], xn[:, sl], xnh[0:C, sl])
            kp = scp.tile([KT, 2, KC], F32, tag="sc", name=f"kp{i}")
            nc.tensor.matmul(kp[0:C, 0, :], lhsT=wk_sb[:, 0, :], rhs=xnh[:, sl],
                             start=True, stop=False, skip_group_check=True)
            nc.tensor.matmul(kp[0:C, 0, :], lhsT=wk_sb[:, 0, :], rhs=xnl[:, sl],
                             start=False, stop=True, skip_group_check=True)
            if i % 2 == 0:
                nc.vector.tensor_copy(kst[0:C, sl], kp[0:C, 0, :])
            else:
                nc.scalar.copy(kst[0:C, sl], kp[0:C, 0, :])
        # fp8 casts off the kp-rotation critical path (chunk-0 pass-1 runs
        # f16, so these only gate chunks 1-3 and have plenty of slack).
        for i in range(NKC):
            sl = slice(i * KC, (i + 1) * KC)
            nc.scalar.copy(k8[:, 0, sl], kst[0:C // 2, sl])
            nc.scalar.copy(k8[:, 1, sl], kst[C // 2:C, sl])
        # residual + output bias for our query half
        xnb = persist.tile([C, HALF], F32)
        nc.vector.tensor_scalar_add(xnb, xn[:, 0:HALF], bo_sb)

        for i in range(NQC):
            sl = slice(i * QC, (i + 1) * QC)
            qp = scp.tile([KT, 2, KC], F32, tag="sc", name=f"qp{i}")
            nc.tensor.matmul(qp[0:C, 0, :], lhsT=wq_sb[:, 0, :], rhs=xnh[:, sl],
                             start=True, stop=False, skip_group_check=True)
            nc.tensor.matmul(qp[0:C, 0, :], lhsT=wq_sb[:, 0, :], rhs=xnl[:, sl],
                             start=False, stop=True, skip_group_check=True)
            if i % 2 == 0:
                nc.vector.tensor_copy(qrhs[0:C, sl], qp[0:C, 0, :])
            else:
                nc.scalar.copy(qrhs[0:C, sl], qp[0:C, 0, :])
            if i > 0:  # chunk 0 pass-1 runs f16; no q8 needed for it
                nc.scalar.copy(q8[:, 0, sl], qrhs[0:C // 2, sl])
                nc.scalar.copy(q8[:, 1, sl], qrhs[C // 2:C, sl])

        # ---- incremental pass-1 (row max of chunk c1's q-tiles) ----
        # One k-chunk matmul + DVE row-max per step; every 8th step finalizes
        # a q-tile's -max into qrhs row 64 via a PE transpose. (DVE can read
        # only ONE PSUM operand per instruction, so per-chunk tensor_reduce
        # it is.)
        p1_state = {}

        def pass1_step(c1):
            st = p1_state.setdefault(c1, {"step": 0})
            step = st["step"]
            if step >= 4 * NKC:
                return
            st["step"] = step + 1
            t4, ci = divmod(step, NKC)
            t = c1 * 4 + t4
            tq = slice(t * QT, (t + 1) * QT)
            if ci == 0:
                st["cm"] = small.tile([QT, NKC], F32, tag="cm",
                                      name=f"cm{t}")
            cm = st["cm"]
            cs = slice(ci * KC, (ci + 1) * KC)
            p1 = p1p.tile([QT, KC], F32, tag="p1", name=f"p1_{t}_{ci}")
            if USE_FP8_P1 and c1 > 0:
                nc.tensor.matmul(p1, lhsT=q8[:, :, tq], rhs=k8[:, :, cs],
                                 perf_mode=mybir.MatmulPerfMode.DoubleRow,
                                 start=True, stop=True, skip_group_check=True)
            else:
                nc.tensor.matmul(p1, lhsT=qrhs[0:C, tq], rhs=kst[0:C, cs],
                                 start=True, stop=True, skip_group_check=True)
            nc.vector.tensor_reduce(cm[:, ci:ci + 1], p1,
                                    axis=mybir.AxisListType.X,
                                    op=mybir.AluOpType.max)
            if ci == NKC - 1:
                nmT = small.tile([QT, 1], F16, tag="nmT", name=f"nmT{t}")
                nc.vector.tensor_reduce(nmT, cm,
                                        axis=mybir.AxisListType.X,
                                        op=mybir.AluOpType.max, negate=True)
                tr = p1p.tile([1, QT], F16, tag="p1", name=f"tr{t}")
                nc.tensor.transpose(tr, nmT, ident)
                nc.scalar.copy(qrhs[C:C + 1, tq], tr[0:1, :])

        # v projection interleaved with chunk-0 pass-1
        for j in range(NKT):
            js = slice(j * KT, (j + 1) * KT)
            vp = p1p.tile([KT, C], F32, tag="p1", name=f"vp{j}")
            nc.tensor.matmul(vp, lhsT=xnh[:, js], rhs=wv_sb[:, 0, :],
                             start=True, stop=False, skip_group_check=True)
            nc.tensor.matmul(vp, lhsT=xnl[:, js], rhs=wv_sb[:, 0, :],
                             start=False, stop=True, skip_group_check=True)
            if j % 2 == 0:
                nc.scalar.copy(vst[:, j, 0:C], vp)
            else:
                nc.vector.tensor_copy(vst[:, j, 0:C], vp)
            pass1_step(0)

        # ---- main loop over q-chunks ----
        # attn@v matmuls trail the score/exp conveyor by AV_LAG pairs so they
        # never head-of-line block the in-order PE queue on a fresh exp.
        ao_aug = persist.tile([C + 1, HALF], BF16)  # rows 0:64 attn@v, 64 denom
        AV_LAG = 3
        av_fifo = []

        def emit_av(c, p, otp, ab):
            for h in range(2):
                j = 2 * p + h
                nc.tensor.matmul(otp, lhsT=vst[:, j, :], rhs=ab[:, h, :],
                                 start=(j == 0), stop=(j == NKT - 1),
                                 skip_group_check=True)

        def emit_epilogue(c, otp):
            qs = slice(c * QC, (c + 1) * QC)
            # single copy moves attn@v rows AND the denominator row; DVE
            # keeps it off the exp-laden ScalarE queue so the fx matmul
            # doesn't head-of-line block the PE behind pending exps.
            # (GPSIMD cannot read PSUM on TRN2.)
            nc.vector.tensor_copy(ao_aug[:, qs], otp[0:C + 1, :])
            # fx rows 0:64 = Wo @ attn@v ; rows 64:128 = denominator bcast
            fx = p1p.tile([KT, QC], F32, tag="p1", name=f"fx{c}")
            nc.tensor.matmul(fx, lhsT=wo_sb, rhs=ao_aug[:, qs],
                             start=True, stop=True, skip_group_check=True)
            ibs = fpool.tile([C, QC], F32, tag="ibs", name=f"ibs{c}")
            if USE_RECIP_APPROX:
                nc.vector.reciprocal_approx_fast(out=ibs, in_=fx[C:2 * C, :])
            else:
                nc.vector.reciprocal(out=ibs, in_=fx[C:2 * C, :])
            fin = fpool.tile([C, QC], F32, tag="fin", name=f"fin{c}")
            nc.vector.tensor_mul(fin, fx[0:C, :], ibs)
            nc.vector.tensor_add(fin, fin, xnb[:, qs])
            eng = nc.sync if c % 2 == 0 else nc.scalar
            eng.dma_start(out=out_d.ap()[:, qs], in_=fin)

        def pop_av():
            c0, p0, otp0, ab0 = av_fifo.pop(0)
            emit_av(c0, p0, otp0, ab0)
            if p0 == NPR - 1:
                emit_epilogue(c0, otp0)

        for c in range(NQC):
            qs = slice(c * QC, (c + 1) * QC)
            otp = avp.tile([66, QC], F32, tag="av", name=f"otp{c}")
            for p in range(NPR):
                sc = scp.tile([KT, 2, QC], F32, tag="sc", name=f"sc{c}_{p}")
                for h in range(2):
                    js = slice((2 * p + h) * KT, (2 * p + h + 1) * KT)
                    nc.tensor.matmul(sc[:, h, :], lhsT=kst[:, js],
                                     rhs=qrhs[:, qs],
                                     start=True, stop=True,
                                     skip_group_check=True)
                ab = apool.tile([KT, 2, QC], BF16, tag="ab", name=f"ab{c}_{p}")
                nc.scalar.activation(out=ab.rearrange("p a b -> p (a b)"),
                                     in_=sc.rearrange("p a b -> p (a b)"),
                                     func=mybir.ActivationFunctionType.Exp,
                                     bias=0.0, scale=1.0)
                av_fifo.append((c, p, otp, ab))
                if len(av_fifo) > AV_LAG:
                    pop_av()
                if c + 1 < NQC:
                    # 3 steps/pair drains the 32 steps by pair 11 so the
                    # next chunk's -max bias lands well before the boundary.
                    pass1_step(c + 1)
                    pass1_step(c + 1)
                    pass1_step(c + 1)
        while av_fifo:
            pop_av()


def prep_inputs(x, w_qkv, b_qkv, w_out, b_out):
    """Host-side slicing/packing into per-core input maps."""
    x = np.asarray(x, dtype=np.float32).reshape(B, C, N)
    w_qkv = np.asarray(w_qkv, dtype=np.float32)
    b_qkv = np.asarray(b_qkv, dtype=np.float32)
    w_out = np.asarray(w_out, dtype=np.float32)
    b_out = np.asarray(b_out, dtype=np.float32)

    s = float(C) ** 0.5  # reference multiplies scores by sqrt(C)
    wq1 = np.concatenate([s * w_qkv[0:C].T, s * b_qkv[None, 0:C]], axis=0)
    wk1 = np.concatenate([w_qkv[C:2 * C].T, b_qkv[None, C:2 * C]], axis=0)
    wv1 = np.concatenate([w_qkv[2 * C:3 * C].T, b_qkv[None, 2 * C:3 * C]], axis=0)

    def hilo16(w):  # [65, 64] -> [65, 2, 64] f16 (hi, lo), hi+lo ~== w
        hi = w.astype(np.float16)
        lo = (w - hi.astype(np.float32)).astype(np.float16)
        return np.ascontiguousarray(np.stack([hi, lo], axis=1))

    wq1 = hilo16(np.ascontiguousarray(wq1))
    wk1 = hilo16(np.ascontiguousarray(wk1))
    wv1 = hilo16(np.ascontiguousarray(wv1))
    # wo1: [65, 128]; rows 0:64 cols 0:64 = WoT; row 64 cols 64:128 = 1
    # so one K=65 matmul gives [Wo@ao ; denom broadcast] stacked.
    import ml_dtypes
    wo1 = np.zeros((C + 1, KT), dtype=np.float32)
    wo1[0:C, 0:C] = w_out.T
    wo1[C, C:KT] = 1.0
    wo1 = np.ascontiguousarray(wo1).astype(ml_dtypes.bfloat16)
    bo = np.ascontiguousarray(b_out[:, None])

    in_maps = []
    for j in range(NCORES):
        b, h = divmod(j, 2)
        xs = x[b]
        if h == 1:
            xs = np.concatenate([xs[:, HALF:], xs[:, :HALF]], axis=1)
        in_maps.append({
            "x": np.ascontiguousarray(xs),
            "wq1": wq1,
            "wk1": wk1,
            "wv1": wv1,
            "wo1": wo1,
            "bo": bo,
        })
    return in_maps


def gather_output(results):
    out = np.empty((B, C, N), dtype=np.float32)
    for j in range(NCORES):
        b, h = divmod(j, 2)
        out[b][:, h * HALF:(h + 1) * HALF] = results[j]["out"]
    return out.reshape(B, C, H, W)


_NC_CACHE = {}


def get_nc():
    key = "v5"
    if key not in _NC_CACHE:
        _NC_CACHE[key] = build_nc()
    return _NC_CACHE[key]


def kernel(x, w_qkv, b_qkv, w_out, b_out):
    nc = get_nc()
    in_maps = prep_inputs(x, w_qkv, b_qkv, w_out, b_out)
    res = run_bass_kernel_spmd(nc, in_maps, list(range(NCORES)))
    return gather_output(res.results)


# revision 31
# speedup vs baseline: 1.0519x; 1.0437x over previous
"""Trainium2 Bass kernel for an AttentionBlock (InstanceNorm + single-head
spatial self-attention + projection + residual).

Full-input contract: kernel(**inputs) takes the complete tensors and returns
the complete output. Internally shards across 8 NeuronCores: data-parallel
over batch (B=4 -> 4 pairs of cores), sequence-parallel over the N=4096 query
positions within each sample (2 cores per sample, 2048 queries each).

All 8 cores run the *same* program; the query-half assignment is done by
rotating the spatial columns of x host-side (attention and instance-norm
statistics are invariant under column permutation).

v4 design (v3 + engine rebalance):
  - pass-1 ([q,k] layout): f16 matmul pairs per (q-tile, 2 k-chunks); DVE
    tensor_tensor_reduce fuses the elementwise max of the two PSUM tiles
    with the row-max accumulation (half the DVE passes of per-tile reduce).
  - pass-2 ([k,q] layout): K=65 matmul (64 channels + bias row carrying
    -rowmax) produces shifted scores in PSUM; ScalarE exp writes f16
    attention blocks straight to SBUF in the layout attn@v consumes.
  - attn@v: v is augmented with a ones column so softmax denominators fall
    out of the same accumulation.
  - epilogue: one K=65 matmul with wo1=[[WoT,0],[0,1...1]] yields both the
    output projection (rows 0:64) and the denominator broadcast to rows
    64:128; reciprocal+normalize then run at full DVE width.
  - copies ride GpSimd (Pool) where possible to keep ScalarE free for exp.
  - no PE warm-up/dummy matmuls: the HAM power throttle (k=4/8 duty) is
    triggered by sustained PE activity, so wasted matmuls cost twice.
"""

import os
import sys
import numpy as np
from contextlib import ExitStack

for _p in ("/opt/trn_rl_repo", "/root/.axon_site/_ro/trn_rl_repo"):
    if os.path.isdir(_p) and _p not in sys.path:
        sys.path.append(_p)

from concourse import bass, bacc, tile, mybir, masks  # noqa: E402
from concourse.bass_utils import run_bass_kernel_spmd  # noqa: E402

F32 = mybir.dt.float32
F16 = mybir.dt.float16
BF16 = mybir.dt.bfloat16
F8 = mybir.dt.float8e4

B, C, H, W = 4, 64, 64, 64
N = H * W            # 4096 spatial positions (attention length)
HALF = N // 2        # queries per core
KT = 128             # pass-2 k-tile (partition dim of transposed scores)
NKT = N // KT        # 32 k-tiles
NPR = NKT // 2       # 16 k-tile pairs
QC = 512             # q-chunk (PSUM bank free dim)
NQC = HALF // QC     # 4 q-chunks per core
QT = 128             # pass-1 q-tile
KC = 512             # pass-1 k-chunk
NKC = N // KC        # 8
EPS = 1e-5
NCORES = 8
NEG_INF = -3.0e38
USE_FP8_P1 = os.environ.get("USE_FP8_P1", "1") == "1"
USE_RECIP_APPROX = os.environ.get("USE_RECIP_APPROX", "0") == "1"
WARM_N = int(os.environ.get("WARM_N", "12"))


def build_nc():
    nc = bacc.Bacc("TRN2", target_bir_lowering=False, debug=False)

    x_d = nc.dram_tensor("x", [C, N], F32, kind="ExternalInput")
    wq_d = nc.dram_tensor("wq1", [C + 1, 2, C], F16, kind="ExternalInput")
    wk_d = nc.dram_tensor("wk1", [C + 1, 2, C], F16, kind="ExternalInput")
    wv_d = nc.dram_tensor("wv1", [C + 1, 2, C], F16, kind="ExternalInput")
    wo_d = nc.dram_tensor("wo1", [C + 1, KT], BF16, kind="ExternalInput")
    bo_d = nc.dram_tensor("bo", [C, 1], F32, kind="ExternalInput")
    out_d = nc.dram_tensor("out", [C, HALF], F32, kind="ExternalOutput")

    with tile.TileContext(nc) as tc:
        _body(tc, x_d, wq_d, wk_d, wv_d, wo_d, bo_d, out_d)
    nc.compile()
    return nc


def _body(tc, x_d, wq_d, wk_d, wv_d, wo_d, bo_d, out_d):
    nc = tc.nc
    with ExitStack() as ctx:
        persist = ctx.enter_context(tc.tile_pool(name="persist", bufs=1))
        small = ctx.enter_context(tc.tile_pool(name="small", bufs=4))
        apool = ctx.enter_context(tc.tile_pool(name="apool", bufs=6))
        fpool = ctx.enter_context(tc.tile_pool(name="fpool", bufs=2))
        # PSUM budget (8 banks): p1p 2 + scp 4 + avp 2
        p1p = ctx.enter_context(tc.tile_pool(name="p1p", bufs=2, space="PSUM"))
        scp = ctx.enter_context(tc.tile_pool(name="scp", bufs=2, space="PSUM"))
        avp = ctx.enter_context(tc.tile_pool(name="avp", bufs=2, space="PSUM"))

        # ---- inputs ----
        x_sb = persist.tile([C, N], F32)
        wq_sb = persist.tile([C + 1, 2, C], F16)
        nc.sync.dma_start(out=wq_sb, in_=wq_d.ap())
        wk_sb = persist.tile([C + 1, 2, C], F16)
        nc.scalar.dma_start(out=wk_sb, in_=wk_d.ap())
        wv_sb = persist.tile([C + 1, 2, C], F16)
        nc.sync.dma_start(out=wv_sb, in_=wv_d.ap())
        wo_sb = persist.tile([C + 1, KT], BF16)
        nc.scalar.dma_start(out=wo_sb, in_=wo_d.ap())
        bo_sb = persist.tile([C, 1], F32)
        nc.sync.dma_start(out=bo_sb, in_=bo_d.ap())
        eps_t = persist.tile([C, 1], F32)
        nc.vector.memset(eps_t, EPS)
        ident = persist.tile([QT, QT], F16)
        masks.make_identity(nc, ident)
        dmy = persist.tile([QT, KC], F16)
        nc.gpsimd.memset(dmy, 0.25)

        def warm_pe(tag, n):
            """Back-to-back dummy matmuls (no readers -> no stalls) that keep
            the PE queue dense through a full HAM window so the clock gate
            opens to 8/8 (2.4 GHz)."""
            for w in range(n):
                wp = p1p.tile([QT, KC], F32, tag="p1", name=f"wp{tag}_{w}")
                nc.tensor.matmul(wp, lhsT=ident, rhs=dmy,
                                 start=True, stop=True, skip_group_check=True)

        # x load split across queues; bn_stats per chunk as it arrives
        stats = persist.tile([C, NKC, nc.vector.BN_STATS_DIM], F32)
        warm_pe("h", WARM_N)
        for i in range(NKC):
            sl = slice(i * KC, (i + 1) * KC)
            eng = nc.sync if i % 2 == 0 else nc.scalar
            eng.dma_start(out=x_sb[:, sl], in_=x_d.ap()[:, sl])
            nc.vector.bn_stats(out=stats[:, i, :], in_=x_sb[:, sl])
        mv = persist.tile([C, nc.vector.BN_AGGR_DIM], F32)
        nc.vector.bn_aggr(out=mv, in_=stats)
        stdv = persist.tile([C, 1], F32)
        nc.scalar.activation(out=stdv, in_=mv[:, 1:2],
                             func=mybir.ActivationFunctionType.Sqrt,
                             bias=eps_t, scale=1.0)
        rstd = persist.tile([C, 1], F32)
        nc.vector.reciprocal(out=rstd, in_=stdv)
        nmr = persist.tile([C, 1], F32)
        nc.vector.tensor_mul(nmr, mv[:, 0:1], rstd)
        nc.vector.tensor_scalar_mul(nmr, nmr, -1.0)

        # xn (f32, residual + lo-part), f16 hi/lo with bias-row for QKV.
        # Per 512-chunk: xn on DVE, xnh on ScalarE, xnl on GpSimd, then the
        # k-projection for that chunk immediately (keeps the PE fed early).
        xn = persist.tile([C, N], F32)
        xnh = persist.tile([C + 1, N], F16)
        xnl = persist.tile([C + 1, N], F16)
        nc.gpsimd.memset(xnh[C:C + 1, :], 1.0)
        nc.gpsimd.memset(xnl[C:C + 1, :], 0.0)
        kst = persist.tile([C + 1, N], F16)      # rows 0:64 k, row 64 ones
        qrhs = persist.tile([C + 1, HALF], F16)  # rows 0:64 q*sqrt(C), row 64 -max
        nc.gpsimd.memset(kst[C:C + 1, :], 1.0)
        vst = persist.tile([KT, NKT, 66], BF16)  # [kpos, ktile, 64 v + ones + pad]
        nc.gpsimd.memset(vst[:, :, 64:65], 1.0)
        nc.gpsimd.memset(vst[:, :, 65:66], 0.0)
        # fp8 copies of k and q*sqrt(C), channel-split [32, 2, *] for the
        # DoubleRow pass-1 matmuls (2 contraction sub-tiles per pass).
        k8 = persist.tile([C // 2, 2, N], F8)
        q8 = persist.tile([C // 2, 2, HALF], F8)

        for i in range(NKC):
            sl = slice(i * KC, (i + 1) * KC)
            nc.vector.tensor_scalar(out=xn[:, sl], in0=x_sb[:, sl],
                                    scalar1=rstd, scalar2=nmr,
                                    op0=mybir.AluOpType.mult,
                                    op1=mybir.AluOpType.add)
            nc.scalar.activation(out=xnh[0:C, sl], in_=x_sb[:, sl],
                                 func=mybir.ActivationFunctionType.Identity,
                                 bias=nmr, scale=rstd)
            nc.gpsimd.tensor_sub(xnl[0:C, sl], xn[:, sl], xnh[0:C, sl])
            kp = scp.tile([KT, 2, KC], F32, tag="sc", name=f"kp{i}")
            nc.tensor.matmul(kp[0:C, 0, :], lhsT=wk_sb[:, 0, :], rhs=xnh[:, sl],
                             start=True, stop=False, skip_group_check=True)
            nc.tensor.matmul(kp[0:C, 0, :], lhsT=wk_sb[:, 0, :], rhs=xnl[:, sl],
                             start=False, stop=True, skip_group_check=True)
            if i % 2 == 0:
                nc.vector.tensor_copy(kst[0:C, sl], kp[0:C, 0, :])
            else:
                nc.scalar.copy(kst[0:C, sl], kp[0:C, 0, :])
        # fp8 casts off the kp-rotation critical path (chunk-0 pass-1 runs
        # f16, so these only gate chunks 1-3 and have plenty of slack).
        for i in range(NKC):
            sl = slice(i * KC, (i + 1) * KC)
            nc.scalar.copy(k8[:, 0, sl], kst[0:C // 2, sl])
            nc.scalar.copy(k8[:, 1, sl], kst[C // 2:C, sl])
        # residual + output bias for our query half
        xnb = persist.tile([C, HALF], F32)
        nc.vector.tensor_scalar_add(xnb, xn[:, 0:HALF], bo_sb)

        for i in range(NQC):
            sl = slice(i * QC, (i + 1) * QC)
            qp = scp.tile([KT, 2, KC], F32, tag="sc", name=f"qp{i}")
            nc.tensor.matmul(qp[0:C, 0, :], lhsT=wq_sb[:, 0, :], rhs=xnh[:, sl],
                             start=True, stop=False, skip_group_check=True)
            nc.tensor.matmul(qp[0:C, 0, :], lhsT=wq_sb[:, 0, :], rhs=xnl[:, sl],
                             start=False, stop=True, skip_group_check=True)
            if i % 2 == 0:
                nc.vector.tensor_copy(qrhs[0:C, sl], qp[0:C, 0, :])
            else:
                nc.scalar.copy(qrhs[0:C, sl], qp[0:C, 0, :])
            if i > 0:  # chunk 0 pass-1 runs f16; no q8 needed for it
                nc.scalar.copy(q8[:, 0, sl], qrhs[0:C // 2, sl])
                nc.scalar.copy(q8[:, 1, sl], qrhs[C // 2:C, sl])

        # ---- incremental pass-1 (row max of chunk c1's q-tiles) ----
        # One k-chunk matmul + DVE row-max per step; every 8th step finalizes
        # a q-tile's -max into qrhs row 64 via a PE transpose. (DVE can read
        # only ONE PSUM operand per instruction, so per-chunk tensor_reduce
        # it is.)
        p1_state = {}

        def pass1_step(c1):
            st = p1_state.setdefault(c1, {"step": 0})
            step = st["step"]
            if step >= 4 * NKC:
                return
            st["step"] = step + 1
            t4, ci = divmod(step, NKC)
            t = c1 * 4 + t4
            tq = slice(t * QT, (t + 1) * QT)
            if ci == 0:
                st["cm"] = small.tile([QT, NKC], F32, tag="cm",
                                      name=f"cm{t}")
            cm = st["cm"]
            cs = slice(ci * KC, (ci + 1) * KC)
            p1 = p1p.tile([QT, KC], F32, tag="p1", name=f"p1_{t}_{ci}")
            if USE_FP8_P1 and c1 > 0:
                nc.tensor.matmul(p1, lhsT=q8[:, :, tq], rhs=k8[:, :, cs],
                                 perf_mode=mybir.MatmulPerfMode.DoubleRow,
                                 start=True, stop=True, skip_group_check=True)
            else:
                nc.tensor.matmul(p1, lhsT=qrhs[0:C, tq], rhs=kst[0:C, cs],
                                 start=True, stop=True, skip_group_check=True)
            nc.vector.tensor_reduce(cm[:, ci:ci + 1], p1,
                                    axis=mybir.AxisListType.X,
                                    op=mybir.AluOpType.max)
            if ci == NKC - 1:
                nmT = small.tile([QT, 1], F16, tag="nmT", name=f"nmT{t}")
                nc.vector.tensor_reduce(nmT, cm,
                                        axis=mybir.AxisListType.X,
                                        op=mybir.AluOpType.max, negate=True)
                tr = p1p.tile([1, QT], F16, tag="p1", name=f"tr{t}")
                nc.tensor.transpose(tr, nmT, ident)
                nc.scalar.copy(qrhs[C:C + 1, tq], tr[0:1, :])

        # v projection interleaved with chunk-0 pass-1
        for j in range(NKT):
            js = slice(j * KT, (j + 1) * KT)
            vp = p1p.tile([KT, C], F32, tag="p1", name=f"vp{j}")
            nc.tensor.matmul(vp, lhsT=xnh[:, js], rhs=wv_sb[:, 0, :],
                             start=True, stop=False, skip_group_check=True)
            nc.tensor.matmul(vp, lhsT=xnl[:, js], rhs=wv_sb[:, 0, :],
                             start=False, stop=True, skip_group_check=True)
            if j % 2 == 0:
                nc.scalar.copy(vst[:, j, 0:C], vp)
            else:
                nc.vector.tensor_copy(vst[:, j, 0:C], vp)
            pass1_step(0)

        # ---- main loop over q-chunks ----
        # attn@v matmuls trail the score/exp conveyor by AV_LAG pairs so they
        # never head-of-line block the in-order PE queue on a fresh exp.
        ao_aug = persist.tile([C + 1, HALF], BF16)  # rows 0:64 attn@v, 64 denom
        AV_LAG = 3
        av_fifo = []

        def emit_av(c, p, otp, ab):
            for h in range(2):
                j = 2 * p + h
                nc.tensor.matmul(otp, lhsT=vst[:, j, :], rhs=ab[:, h, :],
                                 start=(j == 0), stop=(j == NKT - 1),
                                 skip_group_check=True)

        def emit_epilogue(c, otp):
            qs = slice(c * QC, (c + 1) * QC)
            # single copy moves attn@v rows AND the denominator row; DVE
            # keeps it off the exp-laden ScalarE queue so the fx matmul
            # doesn't head-of-line block the PE behind pending exps.
            # (GPSIMD cannot read PSUM on TRN2.)
            nc.vector.tensor_copy(ao_aug[:, qs], otp[0:C + 1, :])
            # fx rows 0:64 = Wo @ attn@v ; rows 64:128 = denominator bcast
            fx = p1p.tile([KT, QC], F32, tag="p1", name=f"fx{c}")
            nc.tensor.matmul(fx, lhsT=wo_sb, rhs=ao_aug[:, qs],
                             start=True, stop=True, skip_group_check=True)
            ibs = fpool.tile([C, QC], F32, tag="ibs", name=f"ibs{c}")
            if USE_RECIP_APPROX:
                nc.vector.reciprocal_approx_fast(out=ibs, in_=fx[C:2 * C, :])
            else:
                nc.vector.reciprocal(out=ibs, in_=fx[C:2 * C, :])
            fin = fpool.tile([C, QC], F32, tag="fin", name=f"fin{c}")
            nc.vector.tensor_mul(fin, fx[0:C, :], ibs)
            nc.vector.tensor_add(fin, fin, xnb[:, qs])
            eng = nc.sync if c % 2 == 0 else nc.scalar
            eng.dma_start(out=out_d.ap()[:, qs], in_=fin)

        def pop_av():
            c0, p0, otp0, ab0 = av_fifo.pop(0)
            emit_av(c0, p0, otp0, ab0)
            if p0 == NPR - 1:
                emit_epilogue(c0, otp0)

        for c in range(NQC):
            qs = slice(c * QC, (c + 1) * QC)
            otp = avp.tile([66, QC], F32, tag="av", name=f"otp{c}")
            for p in range(NPR):
                sc = scp.tile([KT, 2, QC], F32, tag="sc", name=f"sc{c}_{p}")
                for h in range(2):
                    js = slice((2 * p + h) * KT, (2 * p + h + 1) * KT)
                    nc.tensor.matmul(sc[:, h, :], lhsT=kst[:, js],
                                     rhs=qrhs[:, qs],
                                     start=True, stop=True,
                                     skip_group_check=True)
                ab = apool.tile([KT, 2, QC], BF16, tag="ab", name=f"ab{c}_{p}")
                nc.scalar.activation(out=ab.rearrange("p a b -> p (a b)"),
                                     in_=sc.rearrange("p a b -> p (a b)"),
                                     func=mybir.ActivationFunctionType.Exp,
                                     bias=0.0, scale=1.0)
                av_fifo.append((c, p, otp, ab))
                if len(av_fifo) > AV_LAG:
                    pop_av()
                if c + 1 < NQC:
                    # 3 steps/pair drains the 32 steps by pair 11 so the
                    # next chunk's -max bias lands well before the boundary.
                    pass1_step(c + 1)
                    pass1_step(c + 1)
                    pass1_step(c + 1)
        while av_fifo:
            pop_av()


def prep_inputs(x, w_qkv, b_qkv, w_out, b_out):
    """Host-side slicing/packing into per-core input maps."""
    x = np.asarray(x, dtype=np.float32).reshape(B, C, N)
    w_qkv = np.asarray(w_qkv, dtype=np.float32)
    b_qkv = np.asarray(b_qkv, dtype=np.float32)
    w_out = np.asarray(w_out, dtype=np.float32)
    b_out = np.asarray(b_out, dtype=np.float32)

    s = float(C) ** 0.5  # reference multiplies scores by sqrt(C)
    wq1 = np.concatenate([s * w_qkv[0:C].T, s * b_qkv[None, 0:C]], axis=0)
    wk1 = np.concatenate([w_qkv[C:2 * C].T, b_qkv[None, C:2 * C]], axis=0)
    wv1 = np.concatenate([w_qkv[2 * C:3 * C].T, b_qkv[None, 2 * C:3 * C]], axis=0)

    def hilo16(w):  # [65, 64] -> [65, 2, 64] f16 (hi, lo), hi+lo ~== w
        hi = w.astype(np.float16)
        lo = (w - hi.astype(np.float32)).astype(np.float16)
        return np.ascontiguousarray(np.stack([hi, lo], axis=1))

    wq1 = hilo16(np.ascontiguousarray(wq1))
    wk1 = hilo16(np.ascontiguousarray(wk1))
    wv1 = hilo16(np.ascontiguousarray(wv1))
    # wo1: [65, 128]; rows 0:64 cols 0:64 = WoT; row 64 cols 64:128 = 1
    # so one K=65 matmul gives [Wo@ao ; denom broadcast] stacked.
    import ml_dtypes
    wo1 = np.zeros((C + 1, KT), dtype=np.float32)
    wo1[0:C, 0:C] = w_out.T
    wo1[C, C:KT] = 1.0
    wo1 = np.ascontiguousarray(wo1).astype(ml_dtypes.bfloat16)
    bo = np.ascontiguousarray(b_out[:, None])

    in_maps = []
    for j in range(NCORES):
        b, h = divmod(j, 2)
        xs = x[b]
        if h == 1:
            xs = np.concatenate([xs[:, HALF:], xs[:, :HALF]], axis=1)
        in_maps.append({
            "x": np.ascontiguousarray(xs),
            "wq1": wq1,
            "wk1": wk1,
            "wv1": wv1,
            "wo1": wo1,
            "bo": bo,
        })
    return in_maps


def gather_output(results):
    out = np.empty((B, C, N), dtype=np.float32)
    for j in range(NCORES):
        b, h = divmod(j, 2)
        out[b][:, h * HALF:(h + 1) * HALF] = results[j]["out"]
    return out.reshape(B, C, H, W)


_NC_CACHE = {}


def get_nc():
    key = "v5"
    if key not in _NC_CACHE:
        _NC_CACHE[key] = build_nc()
    return _NC_CACHE[key]


def kernel(x, w_qkv, b_qkv, w_out, b_out):
    nc = get_nc()
    in_maps = prep_inputs(x, w_qkv, b_qkv, w_out, b_out)
    res = run_bass_kernel_spmd(nc, in_maps, list(range(NCORES)))
    return gather_output(res.results)
